# revision 4
# baseline (speedup 1.0000x reference)
"""Binary-weight 3x3 conv via 1D Winograd F(2,3) along H on 8 TRN2 cores.

Data-parallel over batch (4 images/core). The y-axis 3-tap conv becomes
Winograd F(2,3): 4 transformed products per 2 output rows (1.5x less PE
work than direct); the x-axis stays a direct 3-tap conv folded into the
matmul accumulation: per 7-tile output block, 6 accumulating bf16
matmuls (2 cin tiles x 3 x-taps) against contiguous windows of the
transformed input V. F(2,3)'s transforms are all +-1 combos (8 DVE ops
per image each way), leaving the vector engine far below the tensor
engine, which streams matmuls back to back at ~170ns.

Layout: rows are pitch 58 = [56 data][2 zero pads] (116B = 4B-aligned,
so every transform AP runs in the fast packed DVE modes); the conv
window for x-tap kw starts one element before the row, reading the
previous row's trailing zeros as the left pad; each V plane carries 2
leading zero guard slots for the very first window. PSUM banks hold 7
tile-rows (N=404, only 2 junk columns per row).

Precision: matmul operands bf16 (fp16 matmuls pace ~20% slower on the
PE), PSUM fp32, drains/staging fp16 -> rel err ~3e-3. Input casts
fp32->bf16 inside the SWDGE DMA; image 0's first cin-half goes
HWDGE+DVE-cast to shave startup; output casts fp16->fp32 inside the
SWDGE DMA. ~30 tiny warm-up matmuls hold the PE clock at 2.4GHz before
real work lands; the last image's inverse+store is split so only a
quarter image trails the final matmul.
"""

import numpy as np

N_CORES = 8
B_PER_CORE = 4  # 32 images / 8 cores
CIN = 256
COUT = 256
H = W = 56
TY = 28  # y tiles of 2 output rows
PITCH = 58  # row pitch: 56 data + 2 trailing zero pads (116B, 4B-aligned)
XROWS = 58  # padded input rows y=-1..56
NK = 4  # Winograd F(2,3) products
NFREE = 6 * PITCH + 56  # 404 = 7 ty-rows per PSUM bank
PSW = 7 * PITCH  # psum tile width 406
VPLANE = 2 + TY * PITCH  # 1626: 2 zero guard slots + 28 rows
TYBLKS = [(0, 7), (7, 7), (14, 7), (21, 7)]

_CACHED = {}


def _build_nc():
    import concourse.mybir as mybir
    from concourse import bacc
    from concourse.tile import TileContext, add_dep_helper
    from concourse.alu_op_type import AluOpType

    f32 = mybir.dt.float32
    f16 = mybir.dt.float16
    bf = mybir.dt.bfloat16
    i32 = mybir.dt.int32
    ADD, SUB = AluOpType.add, AluOpType.subtract

    nc = bacc.Bacc("TRN2", target_bir_lowering=False, debug=False)
    xs = nc.dram_tensor("xs", [B_PER_CORE, CIN, H, W], f32, kind="ExternalInput").ap()
    wt = nc.dram_tensor("wt", [2, 2, 128, NK, 3, 128], bf, kind="ExternalInput").ap()
    out = nc.dram_tensor(
        "out", [B_PER_CORE, COUT, H, W], f32, kind="ExternalOutput"
    ).ap()

    with TileContext(nc) as tc:
        with (
            tc.tile_pool(name="wp", bufs=1) as wp,
            tc.tile_pool(name="xq", bufs=1) as xq,
            tc.tile_pool(name="tp", bufs=2) as tp,
            tc.tile_pool(name="mp", bufs=2) as mp,
            tc.tile_pool(name="yp", bufs=2) as yp,
            tc.tile_pool(name="pp", bufs=7, space="PSUM") as pp,
            tc.tile_pool(name="pw", bufs=1, space="PSUM") as pw,
        ):
            w_sb = wp.tile([128, 2, 2, NK, 3, 128], bf, name="w_sb")
            zf = wp.tile([128, 240], f32, name="zf")
            zb = wp.tile([128, 240], bf, name="zb")
            nc.vector.memset(zf[:], 0.0)
            nc.vector.tensor_copy(out=zb[:], in_=zf[:])


            xb = [
                wp.tile([128, 2 * XROWS * PITCH], bf, name=f"xb{s}")
                for s in range(2)
            ]
            vt = [
                wp.tile([128, 2 * NK * VPLANE], bf, name=f"v{s}")
                for s in range(2)
            ]

            # matmuls on junk data keep the PE clock warm until real work
            # lands (~13us); vt[1] is only written from image 1 onward
            wm = pw.tile([128, PSW], f32, name="wm")
            for _ in range(26):
                nc.tensor.matmul(
                    wm[:, 0:NFREE], lhsT=zb[:, 0:128],
                    rhs=vt[1][:, 0:NFREE],
                    start=True, stop=True, skip_group_check=True,
                )

            def xin(t, img, cit, h0, hn):
                # SWDGE casts fp32->bf16 in the DMA datapath; packed rows
                src = xs[img, cit * 128 : (cit + 1) * 128].rearrange(
                    "p h w -> p (h w)"
                )[:, h0 * W : (h0 + hn) * W]
                return nc.gpsimd.dma_start(
                    out=t[:, cit, h0 * W : (h0 + hn) * W], in_=src
                )

            def xpk_tile():
                return xq.tile([128, 2, H * W], bf, name="xpk", tag="xpk")

            # image 0 arrives in wave-sized row chunks (both cin tiles per
            # chunk first) so transform and matmuls start as early as
            # possible; later chunks are dep-gated off the critical path
            QCH = ((0, 15), (13, 16), (27, 16), (41, 15))  # x-row chunks
            xpk0 = xpk_tile()
            q0ops = [xin(xpk0, 0, cit, *QCH[0]) for cit in range(2)]
            # first-wave weights and the q1 chunks release only after the
            # q0 chunks land so those get the full HBM bandwidth
            for op in (
                nc.sync.dma_start(out=w_sb[:, 0, 0], in_=wt[0, 0]),
                nc.scalar.dma_start(out=w_sb[:, 1, 0], in_=wt[1, 0]),
                xin(xpk0, 0, 0, *QCH[1]),
                xin(xpk0, 0, 1, *QCH[1]),
            ):
                add_dep_helper(
                    op.ins, q0ops[1].ins, sync=False,
                    reason="q0 chunks get full HBM bandwidth",
                )

            # zero pads of the persistent xb tiles and V guard slots once;
            # slot 0 now (image 0 needs it), slot 1 off the critical path
            def init_slot(s):
                t4 = xb[s][:].rearrange(
                    "p (i r c) -> p i r c", r=XROWS, c=PITCH
                )
                nc.vector.tensor_copy(
                    out=t4[:, :, 0 : XROWS : XROWS - 1, :],
                    in_=zb[:, 0 : 4 * PITCH].rearrange(
                        "p (i a c) -> p i a c", i=2, c=PITCH
                    ),
                )
                z2 = zb[:, 0:232].rearrange("p (i a b) -> p i a b", i=2, b=2)
                nc.vector.tensor_copy(out=t4[:, :, 0:58, 56:58], in_=z2)
                nc.vector.tensor_copy(
                    out=vt[s][:].rearrange("p (g q) -> p g q", q=VPLANE)[
                        :, :, 0:2
                    ],
                    in_=zb[:, 0:16].rearrange("p (g b) -> p g b", b=2),
                )

            init_slot(0)

            def repitch(xpk, s, cit, h0, hn):
                # packed [56,56] -> pitch-58 rows as int32 pairs (4B aligned)
                d4 = xb[s][:].bitcast(i32).rearrange(
                    "p (i r c) -> p i r c", r=XROWS, c=PITCH // 2
                )
                s3 = xpk[:].bitcast(i32).rearrange(
                    "p i (r c) -> p i r c", c=W // 2
                )
                nc.vector.tensor_copy(
                    out=d4[:, cit, 1 + h0 : 1 + h0 + hn, 0 : W // 2],
                    in_=s3[:, cit, h0 : h0 + hn, :],
                )

            def vsl(s, cit, k, ty0, tn):
                r0 = (cit * NK + k) * VPLANE + 2 + ty0 * PITCH
                return vt[s][:, r0 : r0 + tn * PITCH].rearrange(
                    "p (t c) -> p t c", c=PITCH
                )

            def fwd(s, cit, ty0, tn):
                # V = BT d: 4 two-term combos, all +-1 coefficients
                x4 = xb[s][:].rearrange("p (i r c) -> p i r c", r=XROWS, c=PITCH)

                def dr(r):
                    lo = 2 * ty0 + r
                    return x4[:, cit, lo : lo + 2 * (tn - 1) + 1 : 2, :]

                tt = nc.vector.tensor_tensor
                tt(out=vsl(s, cit, 0, ty0, tn), in0=dr(0), in1=dr(2), op=SUB)
                tt(out=vsl(s, cit, 1, ty0, tn), in0=dr(1), in1=dr(2), op=ADD)
                tt(out=vsl(s, cit, 2, ty0, tn), in0=dr(2), in1=dr(1), op=SUB)
                return tt(
                    out=vsl(s, cit, 3, ty0, tn), in0=dr(1), in1=dr(3), op=SUB
                )

            def wino_ct(s, ct, m):
                vflat = vt[s]
                for ty0, tb in TYBLKS:
                    for k in range(NK):
                        ps = pp.tile([128, PSW], f32, name="ps", tag="ps")
                        idx = 0
                        for cit in range(2):
                            for kw in range(3):
                                st = (cit * NK + k) * VPLANE + 1 + ty0 * PITCH + kw
                                nc.tensor.matmul(
                                    ps[:, 0:NFREE],
                                    lhsT=w_sb[:, cit, ct, k, kw, :],
                                    rhs=vflat[:, st : st + NFREE],
                                    start=(idx == 0),
                                    stop=(idx == 5),
                                )
                                idx += 1
                        base = ((ct * NK + k) * TY + ty0) * PITCH
                        nc.scalar.copy(
                            out=m[:, base : base + NFREE], in_=ps[:, 0:NFREE]
                        )

            def inv(img, m):
                # y0 = m0+m1+m2 ; y1 = m1-m2-m3 (both cts, whole image)
                m5 = m[:].rearrange(
                    "p (i k t c) -> p i k t c", k=NK, t=TY, c=PITCH
                )
                tt = nc.vector.tensor_tensor
                u = tp.tile([128, 2 * TY * PITCH], f16, name="u", tag="tt")
                v = tp.tile([128, 2 * TY * PITCH], f16, name="v", tag="tt")
                u4 = u[:].rearrange("p (i t c) -> p i t c", t=TY, c=PITCH)
                v4 = v[:].rearrange("p (i t c) -> p i t c", t=TY, c=PITCH)
                tt(out=u4, in0=m5[:, :, 0], in1=m5[:, :, 1], op=ADD)
                tt(out=v4, in0=m5[:, :, 1], in1=m5[:, :, 2], op=SUB)
                for ct in range(2):
                    y = yp.tile([128, H * W], f16, name="y", tag="y")
                    yr = y[:].rearrange("p (r c) -> p r c", c=W)
                    tt(
                        out=yr[:, 0 : H : 2, :],
                        in0=u4[:, ct, :, 0:56],
                        in1=m5[:, ct, 2, :, 0:56],
                        op=ADD,
                    )
                    tt(
                        out=yr[:, 1 : H : 2, :],
                        in0=v4[:, ct, :, 0:56],
                        in1=m5[:, ct, 3, :, 0:56],
                        op=SUB,
                    )
                    # SWDGE casts fp16->fp32 on the way out
                    nc.gpsimd.dma_start(
                        out=out[img, ct * 128 : (ct + 1) * 128].rearrange(
                            "p h w -> p (h w)"
                        ),
                        in_=y[:],
                    )

            def inv_part(img, m, ct, ty0, tn, y):
                # per-ct, per-ty-range inverse + store (tail overlap)
                m5 = m[:].rearrange(
                    "p (i k t c) -> p i k t c", k=NK, t=TY, c=PITCH
                )
                tt = nc.vector.tensor_tensor
                u = tp.tile([128, 2 * TY * PITCH], f16, name="u", tag="tt")
                u4 = u[:].rearrange("p (i t c) -> p i t c", t=TY, c=PITCH)
                ts = slice(ty0, ty0 + tn)
                yr = y[:].rearrange("p (r c) -> p r c", c=W)
                tt(out=u4[:, 0, ts, :], in0=m5[:, ct, 0, ts, :],
                   in1=m5[:, ct, 1, ts, :], op=ADD)
                tt(out=u4[:, 1, ts, :], in0=m5[:, ct, 1, ts, :],
                   in1=m5[:, ct, 2, ts, :], op=SUB)
                tt(out=yr[:, 2 * ty0 : 2 * (ty0 + tn) : 2, :],
                   in0=u4[:, 0, ts, 0:56], in1=m5[:, ct, 2, ts, 0:56], op=ADD)
                tt(out=yr[:, 2 * ty0 + 1 : 2 * (ty0 + tn) : 2, :],
                   in0=u4[:, 1, ts, 0:56], in1=m5[:, ct, 3, ts, 0:56], op=SUB)
                nc.gpsimd.dma_start(
                    out=out[img, ct * 128 : (ct + 1) * 128].rearrange(
                        "p h w -> p (h w)"
                    )[:, 2 * ty0 * W : 2 * (ty0 + tn) * W],
                    in_=y[:, 2 * ty0 * W : 2 * (ty0 + tn) * W],
                )

            # ---- pipeline ----
            pending = []
            xpks = {0: xpk0}
            last = B_PER_CORE - 1
            for img in range(B_PER_CORE):
                s = img % 2
                lastf = None
                if img == 0:
                    for q in range(4):
                        h0, hn = QCH[q]
                        for cit in range(2):
                            repitch(xpks[0], s, cit, h0, hn)
                            lastf = fwd(s, cit, 7 * q, 7)
                        if q == 0:
                            # critical loads done: release the lower-half
                            # chunks and the ct=1 weights
                            for op in (
                                xin(xpk0, 0, 0, *QCH[2]),
                                xin(xpk0, 0, 1, *QCH[2]),
                                xin(xpk0, 0, 0, *QCH[3]),
                                xin(xpk0, 0, 1, *QCH[3]),
                                nc.sync.dma_start(
                                    out=w_sb[:, 0, 1], in_=wt[0, 1]
                                ),
                                nc.scalar.dma_start(
                                    out=w_sb[:, 1, 1], in_=wt[1, 1]
                                ),
                            ):
                                add_dep_helper(
                                    op.ins, lastf.ins, sync=False,
                                    reason="keep head HBM bw for critical loads",
                                )
                    init_slot(1)
                else:
                    for cit in range(2):
                        repitch(xpks[img], s, cit, 0, H)
                        lastf = fwd(s, cit, 0, TY)
                if img + 1 < B_PER_CORE:
                    t = xpk_tile()
                    for cit in range(2):
                        op = xin(t, img + 1, cit, 0, H)
                        if img == 0:
                            add_dep_helper(
                                op.ins, lastf.ins, sync=False,
                                reason="keep head HBM bw for critical loads",
                            )
                    xpks[img + 1] = t
                m = mp.tile([128, 2 * NK * TY * PITCH], f16, name="m", tag="m")
                if img == last:
                    wino_ct(s, 0, m)
                    while pending:
                        inv(*pending.pop(0))
                    y0t = yp.tile([128, H * W], f16, name="y", tag="y")
                    inv_part(img, m, 0, 0, TY, y0t)
                    wino_ct(s, 1, m)
                    y1t = yp.tile([128, H * W], f16, name="y", tag="y")
                    for ty0, tn in TYBLKS:
                        inv_part(img, m, 1, ty0, tn, y1t)
                else:
                    for ct in range(2):
                        wino_ct(s, ct, m)
                    pending.append((img, m))
                    while len(pending) > 1:
                        inv(*pending.pop(0))

    nc.compile()
    return nc


def _get_nc():
    if "nc" not in _CACHED:
        _CACHED["nc"] = _build_nc()
    return _CACHED["nc"]


_G = np.array(
    [[1, 0, 0], [0.5, 0.5, 0.5], [0.5, -0.5, 0.5], [0, 0, 1]],
    dtype=np.float64,
)


def _prep_weights(W_arr):
    import ml_dtypes

    Wb = np.sign(np.asarray(W_arr, dtype=np.float64))  # [co, ci, kh, kw]
    U = np.einsum("kh,oihw->koiw", _G, Wb)  # [4, co, ci, kw]
    U6 = U.reshape(NK, 2, 128, 2, 128, 3)  # k, ct, co, cit, ci, kw
    wt = U6.transpose(3, 1, 4, 0, 5, 2)  # cit, ct, ci, k, kw, co
    return np.ascontiguousarray(wt).astype(ml_dtypes.bfloat16)


def run(x, W, trace=False, trace_kwargs=None):
    from concourse.bass_utils import run_bass_kernel_spmd

    x = np.asarray(x, dtype=np.float32)
    wt = _prep_weights(W)
    nc = _get_nc()
    in_maps = [
        {
            "xs": np.ascontiguousarray(
                x[i * B_PER_CORE : (i + 1) * B_PER_CORE]
            ),
            "wt": wt,
        }
        for i in range(N_CORES)
    ]
    res = run_bass_kernel_spmd(
        nc,
        in_maps,
        list(range(N_CORES)),
        trace=trace,
        trace_kwargs=trace_kwargs or {},
    )
    out = np.concatenate(
        [np.asarray(res.results[i]["out"]) for i in range(N_CORES)]
    )
    return out, res


def kernel(x, W):
    out, _ = run(x, W, trace=False)
    return out


# revision 5
# speedup vs baseline: 1.0242x; 1.0242x over previous
"""Binary-weight 3x3 conv via 1D Winograd F(2,3) along H on 8 TRN2 cores.

Data-parallel over batch (4 images/core). The y-axis 3-tap conv becomes
Winograd F(2,3): 4 transformed products per 2 output rows (1.5x less PE
work than direct); the x-axis stays a direct 3-tap conv folded into the
matmul accumulation: per 7-tile output block, 6 accumulating bf16
matmuls (2 cin tiles x 3 x-taps) against contiguous windows of the
transformed input V. F(2,3)'s transforms are all +-1 combos (8 DVE ops
per image each way), leaving the vector engine far below the tensor
engine, which streams matmuls back to back at ~170ns.

Layout: rows are pitch 58 = [56 data][2 zero pads] (116B = 4B-aligned,
so every transform AP runs in the fast packed DVE modes); the conv
window for x-tap kw starts one element before the row, reading the
previous row's trailing zeros as the left pad; each V plane carries 2
leading zero guard slots for the very first window. PSUM banks hold 7
tile-rows (N=404, only 2 junk columns per row).

Precision: matmul operands bf16 (fp16 matmuls pace ~20% slower on the
PE), PSUM fp32, drains/staging fp16 -> rel err ~3e-3. Input casts
fp32->bf16 inside the SWDGE DMA; image 0's first cin-half goes
HWDGE+DVE-cast to shave startup; output casts fp16->fp32 inside the
SWDGE DMA. ~30 tiny warm-up matmuls hold the PE clock at 2.4GHz before
real work lands; the last image's inverse+store is split so only a
quarter image trails the final matmul.
"""

import numpy as np

N_CORES = 8
B_PER_CORE = 4  # 32 images / 8 cores
CIN = 256
COUT = 256
H = W = 56
TY = 28  # y tiles of 2 output rows
PITCH = 58  # row pitch: 56 data + 2 trailing zero pads (116B, 4B-aligned)
XROWS = 58  # padded input rows y=-1..56
NK = 4  # Winograd F(2,3) products
NFREE = 6 * PITCH + 56  # 404 = 7 ty-rows per PSUM bank
PSW = 7 * PITCH  # psum tile width 406
VPLANE = 2 + TY * PITCH  # 1626: 2 zero guard slots + 28 rows
TYBLKS = [(0, 7), (7, 7), (14, 7), (21, 7)]

_CACHED = {}


def _build_nc():
    import concourse.mybir as mybir
    from concourse import bacc
    from concourse.tile import TileContext, add_dep_helper
    from concourse.alu_op_type import AluOpType

    f32 = mybir.dt.float32
    f16 = mybir.dt.float16
    bf = mybir.dt.bfloat16
    i32 = mybir.dt.int32
    ADD, SUB = AluOpType.add, AluOpType.subtract

    nc = bacc.Bacc("TRN2", target_bir_lowering=False, debug=False)
    xs = nc.dram_tensor("xs", [B_PER_CORE, CIN, H, W], f32, kind="ExternalInput").ap()
    wt = nc.dram_tensor("wt", [2, 2, 128, NK, 3, 128], bf, kind="ExternalInput").ap()
    out = nc.dram_tensor(
        "out", [B_PER_CORE, COUT, H, W], f32, kind="ExternalOutput"
    ).ap()

    with TileContext(nc) as tc:
        with (
            tc.tile_pool(name="wp", bufs=1) as wp,
            tc.tile_pool(name="xq", bufs=1) as xq,
            tc.tile_pool(name="tp", bufs=2) as tp,
            tc.tile_pool(name="mp", bufs=2) as mp,
            tc.tile_pool(name="yp", bufs=2) as yp,
            tc.tile_pool(name="pp", bufs=7, space="PSUM") as pp,
            tc.tile_pool(name="pw", bufs=1, space="PSUM") as pw,
        ):
            w_sb = wp.tile([128, 2, 2, NK, 3, 128], bf, name="w_sb")
            zf = wp.tile([128, 240], f32, name="zf")
            zb = wp.tile([128, 240], bf, name="zb")
            nc.vector.memset(zf[:], 0.0)
            nc.vector.tensor_copy(out=zb[:], in_=zf[:])


            xb = [
                wp.tile([128, 2 * XROWS * PITCH], bf, name=f"xb{s}")
                for s in range(2)
            ]
            vt = [
                wp.tile([128, 2 * NK * VPLANE], bf, name=f"v{s}")
                for s in range(2)
            ]

            # matmuls on junk data keep the PE clock warm until real work
            # lands (~13us); vt[1] is only written from image 1 onward
            wm = pw.tile([128, PSW], f32, name="wm")
            for _ in range(36):
                nc.tensor.matmul(
                    wm[:, 0:NFREE], lhsT=zb[:, 0:128],
                    rhs=vt[1][:, 0:NFREE],
                    start=True, stop=True, skip_group_check=True,
                )

            def xin(t, img, cit, h0, hn):
                # SWDGE casts fp32->bf16 in the DMA datapath; packed rows
                src = xs[img, cit * 128 : (cit + 1) * 128].rearrange(
                    "p h w -> p (h w)"
                )[:, h0 * W : (h0 + hn) * W]
                return nc.gpsimd.dma_start(
                    out=t[:, cit, h0 * W : (h0 + hn) * W], in_=src
                )

            def xpk_tile():
                return xq.tile([128, 2, H * W], bf, name="xpk", tag="xpk")

            # image 0 arrives in wave-sized row chunks (both cin tiles per
            # chunk first) so transform and matmuls start as early as
            # possible; later chunks are dep-gated off the critical path
            QCH = ((0, 15), (13, 16), (27, 16), (41, 15))  # x-row chunks
            xpk0 = xpk_tile()
            q0ops = [xin(xpk0, 0, cit, *QCH[0]) for cit in range(2)]
            for cit in range(2):
                xin(xpk0, 0, cit, *QCH[1])
            # the first-wave weights release only after the q0 chunks land
            # so the transform inputs get the HBM bandwidth first
            for op in (
                nc.sync.dma_start(out=w_sb[:, 0, 0], in_=wt[0, 0]),
                nc.scalar.dma_start(out=w_sb[:, 1, 0], in_=wt[1, 0]),
            ):
                add_dep_helper(
                    op.ins, q0ops[1].ins, sync=False,
                    reason="input chunks get HBM bandwidth first",
                )

            # zero pads of the persistent xb tiles and V guard slots once;
            # slot 0 now (image 0 needs it), slot 1 off the critical path
            def init_slot(s):
                t4 = xb[s][:].rearrange(
                    "p (i r c) -> p i r c", r=XROWS, c=PITCH
                )
                nc.vector.tensor_copy(
                    out=t4[:, :, 0 : XROWS : XROWS - 1, :],
                    in_=zb[:, 0 : 4 * PITCH].rearrange(
                        "p (i a c) -> p i a c", i=2, c=PITCH
                    ),
                )
                z2 = zb[:, 0:232].rearrange("p (i a b) -> p i a b", i=2, b=2)
                nc.vector.tensor_copy(out=t4[:, :, 0:58, 56:58], in_=z2)
                nc.vector.tensor_copy(
                    out=vt[s][:].rearrange("p (g q) -> p g q", q=VPLANE)[
                        :, :, 0:2
                    ],
                    in_=zb[:, 0:16].rearrange("p (g b) -> p g b", b=2),
                )

            init_slot(0)

            def repitch(xpk, s, cit, h0, hn):
                # packed [56,56] -> pitch-58 rows as int32 pairs (4B aligned)
                d4 = xb[s][:].bitcast(i32).rearrange(
                    "p (i r c) -> p i r c", r=XROWS, c=PITCH // 2
                )
                s3 = xpk[:].bitcast(i32).rearrange(
                    "p i (r c) -> p i r c", c=W // 2
                )
                nc.vector.tensor_copy(
                    out=d4[:, cit, 1 + h0 : 1 + h0 + hn, 0 : W // 2],
                    in_=s3[:, cit, h0 : h0 + hn, :],
                )

            def vsl(s, cit, k, ty0, tn):
                r0 = (cit * NK + k) * VPLANE + 2 + ty0 * PITCH
                return vt[s][:, r0 : r0 + tn * PITCH].rearrange(
                    "p (t c) -> p t c", c=PITCH
                )

            def fwd(s, cit, ty0, tn):
                # V = BT d: 4 two-term combos, all +-1 coefficients
                x4 = xb[s][:].rearrange("p (i r c) -> p i r c", r=XROWS, c=PITCH)

                def dr(r):
                    lo = 2 * ty0 + r
                    return x4[:, cit, lo : lo + 2 * (tn - 1) + 1 : 2, :]

                tt = nc.vector.tensor_tensor
                tt(out=vsl(s, cit, 0, ty0, tn), in0=dr(0), in1=dr(2), op=SUB)
                tt(out=vsl(s, cit, 1, ty0, tn), in0=dr(1), in1=dr(2), op=ADD)
                tt(out=vsl(s, cit, 2, ty0, tn), in0=dr(2), in1=dr(1), op=SUB)
                return tt(
                    out=vsl(s, cit, 3, ty0, tn), in0=dr(1), in1=dr(3), op=SUB
                )

            def wino_ct(s, ct, m):
                vflat = vt[s]
                for ty0, tb in TYBLKS:
                    for k in range(NK):
                        ps = pp.tile([128, PSW], f32, name="ps", tag="ps")
                        idx = 0
                        for cit in range(2):
                            for kw in range(3):
                                st = (cit * NK + k) * VPLANE + 1 + ty0 * PITCH + kw
                                nc.tensor.matmul(
                                    ps[:, 0:NFREE],
                                    lhsT=w_sb[:, cit, ct, k, kw, :],
                                    rhs=vflat[:, st : st + NFREE],
                                    start=(idx == 0),
                                    stop=(idx == 5),
                                )
                                idx += 1
                        base = ((ct * NK + k) * TY + ty0) * PITCH
                        nc.scalar.copy(
                            out=m[:, base : base + NFREE], in_=ps[:, 0:NFREE]
                        )

            def inv(img, m):
                # y0 = m0+m1+m2 ; y1 = m1-m2-m3 (both cts, whole image)
                m5 = m[:].rearrange(
                    "p (i k t c) -> p i k t c", k=NK, t=TY, c=PITCH
                )
                tt = nc.vector.tensor_tensor
                u = tp.tile([128, 2 * TY * PITCH], f16, name="u", tag="tt")
                v = tp.tile([128, 2 * TY * PITCH], f16, name="v", tag="tt")
                u4 = u[:].rearrange("p (i t c) -> p i t c", t=TY, c=PITCH)
                v4 = v[:].rearrange("p (i t c) -> p i t c", t=TY, c=PITCH)
                tt(out=u4, in0=m5[:, :, 0], in1=m5[:, :, 1], op=ADD)
                tt(out=v4, in0=m5[:, :, 1], in1=m5[:, :, 2], op=SUB)
                for ct in range(2):
                    y = yp.tile([128, H * W], f16, name="y", tag="y")
                    yr = y[:].rearrange("p (r c) -> p r c", c=W)
                    tt(
                        out=yr[:, 0 : H : 2, :],
                        in0=u4[:, ct, :, 0:56],
                        in1=m5[:, ct, 2, :, 0:56],
                        op=ADD,
                    )
                    tt(
                        out=yr[:, 1 : H : 2, :],
                        in0=v4[:, ct, :, 0:56],
                        in1=m5[:, ct, 3, :, 0:56],
                        op=SUB,
                    )
                    # SWDGE casts fp16->fp32 on the way out
                    nc.gpsimd.dma_start(
                        out=out[img, ct * 128 : (ct + 1) * 128].rearrange(
                            "p h w -> p (h w)"
                        ),
                        in_=y[:],
                    )

            def inv_part(img, m, ct, ty0, tn, y):
                # per-ct, per-ty-range inverse + store (tail overlap)
                m5 = m[:].rearrange(
                    "p (i k t c) -> p i k t c", k=NK, t=TY, c=PITCH
                )
                tt = nc.vector.tensor_tensor
                u = tp.tile([128, 2 * TY * PITCH], f16, name="u", tag="tt")
                u4 = u[:].rearrange("p (i t c) -> p i t c", t=TY, c=PITCH)
                ts = slice(ty0, ty0 + tn)
                yr = y[:].rearrange("p (r c) -> p r c", c=W)
                tt(out=u4[:, 0, ts, :], in0=m5[:, ct, 0, ts, :],
                   in1=m5[:, ct, 1, ts, :], op=ADD)
                tt(out=u4[:, 1, ts, :], in0=m5[:, ct, 1, ts, :],
                   in1=m5[:, ct, 2, ts, :], op=SUB)
                tt(out=yr[:, 2 * ty0 : 2 * (ty0 + tn) : 2, :],
                   in0=u4[:, 0, ts, 0:56], in1=m5[:, ct, 2, ts, 0:56], op=ADD)
                tt(out=yr[:, 2 * ty0 + 1 : 2 * (ty0 + tn) : 2, :],
                   in0=u4[:, 1, ts, 0:56], in1=m5[:, ct, 3, ts, 0:56], op=SUB)
                nc.gpsimd.dma_start(
                    out=out[img, ct * 128 : (ct + 1) * 128].rearrange(
                        "p h w -> p (h w)"
                    )[:, 2 * ty0 * W : 2 * (ty0 + tn) * W],
                    in_=y[:, 2 * ty0 * W : 2 * (ty0 + tn) * W],
                )

            # ---- pipeline ----
            pending = []
            xpks = {0: xpk0}
            last = B_PER_CORE - 1
            for img in range(B_PER_CORE):
                s = img % 2
                lastf = None
                if img == 0:
                    for q in range(4):
                        h0, hn = QCH[q]
                        for cit in range(2):
                            repitch(xpks[0], s, cit, h0, hn)
                            lastf = fwd(s, cit, 7 * q, 7)
                        if q == 0:
                            # critical loads done: release the lower-half
                            # chunks and the ct=1 weights
                            for op in (
                                xin(xpk0, 0, 0, *QCH[2]),
                                xin(xpk0, 0, 1, *QCH[2]),
                                xin(xpk0, 0, 0, *QCH[3]),
                                xin(xpk0, 0, 1, *QCH[3]),
                                nc.sync.dma_start(
                                    out=w_sb[:, 0, 1], in_=wt[0, 1]
                                ),
                                nc.scalar.dma_start(
                                    out=w_sb[:, 1, 1], in_=wt[1, 1]
                                ),
                            ):
                                add_dep_helper(
                                    op.ins, lastf.ins, sync=False,
                                    reason="keep head HBM bw for critical loads",
                                )
                    init_slot(1)
                else:
                    for cit in range(2):
                        repitch(xpks[img], s, cit, 0, H)
                        lastf = fwd(s, cit, 0, TY)
                if img + 1 < B_PER_CORE:
                    t = xpk_tile()
                    for cit in range(2):
                        op = xin(t, img + 1, cit, 0, H)
                        if img == 0:
                            add_dep_helper(
                                op.ins, lastf.ins, sync=False,
                                reason="keep head HBM bw for critical loads",
                            )
                    xpks[img + 1] = t
                m = mp.tile([128, 2 * NK * TY * PITCH], f16, name="m", tag="m")
                if img == last:
                    wino_ct(s, 0, m)
                    while pending:
                        inv(*pending.pop(0))
                    y0t = yp.tile([128, H * W], f16, name="y", tag="y")
                    inv_part(img, m, 0, 0, TY, y0t)
                    wino_ct(s, 1, m)
                    y1t = yp.tile([128, H * W], f16, name="y", tag="y")
                    for ty0, tn in TYBLKS:
                        inv_part(img, m, 1, ty0, tn, y1t)
                else:
                    for ct in range(2):
                        wino_ct(s, ct, m)
                    pending.append((img, m))
                    while len(pending) > 1:
                        inv(*pending.pop(0))

    nc.compile()
    return nc


def _get_nc():
    if "nc" not in _CACHED:
        _CACHED["nc"] = _build_nc()
    return _CACHED["nc"]


_G = np.array(
    [[1, 0, 0], [0.5, 0.5, 0.5], [0.5, -0.5, 0.5], [0, 0, 1]],
    dtype=np.float64,
)


def _prep_weights(W_arr):
    import ml_dtypes

    Wb = np.sign(np.asarray(W_arr, dtype=np.float64))  # [co, ci, kh, kw]
    U = np.einsum("kh,oihw->koiw", _G, Wb)  # [4, co, ci, kw]
    U6 = U.reshape(NK, 2, 128, 2, 128, 3)  # k, ct, co, cit, ci, kw
    wt = U6.transpose(3, 1, 4, 0, 5, 2)  # cit, ct, ci, k, kw, co
    return np.ascontiguousarray(wt).astype(ml_dtypes.bfloat16)


def run(x, W, trace=False, trace_kwargs=None):
    from concourse.bass_utils import run_bass_kernel_spmd

    x = np.asarray(x, dtype=np.float32)
    wt = _prep_weights(W)
    nc = _get_nc()
    in_maps = [
        {
            "xs": np.ascontiguousarray(
                x[i * B_PER_CORE : (i + 1) * B_PER_CORE]
            ),
            "wt": wt,
        }
        for i in range(N_CORES)
    ]
    res = run_bass_kernel_spmd(
        nc,
        in_maps,
        list(range(N_CORES)),
        trace=trace,
        trace_kwargs=trace_kwargs or {},
    )
    out = np.concatenate(
        [np.asarray(res.results[i]["out"]) for i in range(N_CORES)]
    )
    return out, res


def kernel(x, W):
    out, _ = run(x, W, trace=False)
    return out


# revision 6
# speedup vs baseline: 1.0366x; 1.0122x over previous
"""Binary-weight 3x3 conv via 1D Winograd F(2,3) along H on 8 TRN2 cores.

Data-parallel over batch (4 images/core). The y-axis 3-tap conv becomes
Winograd F(2,3): 4 transformed products per 2 output rows (1.5x less PE
work than direct); the x-axis stays a direct 3-tap conv folded into the
matmul accumulation: per 7-tile output block, 6 accumulating bf16
matmuls (2 cin tiles x 3 x-taps) against contiguous windows of the
transformed input V. F(2,3)'s transforms are all +-1 combos (8 DVE ops
per image each way), leaving the vector engine far below the tensor
engine, which streams matmuls back to back at ~170ns.

Layout: rows are pitch 58 = [56 data][2 zero pads] (116B = 4B-aligned,
so every transform AP runs in the fast packed DVE modes); the conv
window for x-tap kw starts one element before the row, reading the
previous row's trailing zeros as the left pad; each V plane carries 2
leading zero guard slots for the very first window. PSUM banks hold 7
tile-rows (N=404, only 2 junk columns per row).

Precision: matmul operands bf16 (fp16 matmuls pace ~20% slower on the
PE), PSUM fp32, drains/staging fp16 -> rel err ~3e-3. Input casts
fp32->bf16 inside the SWDGE DMA; image 0's first cin-half goes
HWDGE+DVE-cast to shave startup; output casts fp16->fp32 inside the
SWDGE DMA. ~30 tiny warm-up matmuls hold the PE clock at 2.4GHz before
real work lands; the last image's inverse+store is split so only a
quarter image trails the final matmul.
"""

import numpy as np

N_CORES = 8
B_PER_CORE = 4  # 32 images / 8 cores
CIN = 256
COUT = 256
H = W = 56
TY = 28  # y tiles of 2 output rows
PITCH = 58  # row pitch: 56 data + 2 trailing zero pads (116B, 4B-aligned)
XROWS = 58  # padded input rows y=-1..56
NK = 4  # Winograd F(2,3) products
NFREE = 6 * PITCH + 56  # 404 = 7 ty-rows per PSUM bank
PSW = 7 * PITCH  # psum tile width 406
VPLANE = 2 + TY * PITCH  # 1626: 2 zero guard slots + 28 rows
TYBLKS = [(0, 7), (7, 7), (14, 7), (21, 7)]

_CACHED = {}


def _build_nc():
    import concourse.mybir as mybir
    from concourse import bacc
    from concourse.tile import TileContext, add_dep_helper
    from concourse.alu_op_type import AluOpType

    f32 = mybir.dt.float32
    f16 = mybir.dt.float16
    bf = mybir.dt.bfloat16
    i32 = mybir.dt.int32
    ADD, SUB = AluOpType.add, AluOpType.subtract

    nc = bacc.Bacc("TRN2", target_bir_lowering=False, debug=False)
    xs = nc.dram_tensor("xs", [B_PER_CORE, CIN, H, W], f32, kind="ExternalInput").ap()
    wt = nc.dram_tensor("wt", [2, 2, 128, NK, 3, 128], bf, kind="ExternalInput").ap()
    out = nc.dram_tensor(
        "out", [B_PER_CORE, COUT, H, W], f32, kind="ExternalOutput"
    ).ap()

    with TileContext(nc) as tc:
        with (
            tc.tile_pool(name="wp", bufs=1) as wp,
            tc.tile_pool(name="xq", bufs=1) as xq,
            tc.tile_pool(name="tp", bufs=2) as tp,
            tc.tile_pool(name="mp", bufs=2) as mp,
            tc.tile_pool(name="yp", bufs=2) as yp,
            tc.tile_pool(name="pp", bufs=7, space="PSUM") as pp,
            tc.tile_pool(name="pw", bufs=1, space="PSUM") as pw,
        ):
            w_sb = wp.tile([128, 2, 2, NK, 3, 128], bf, name="w_sb")
            zf = wp.tile([128, 240], f32, name="zf")
            zb = wp.tile([128, 240], bf, name="zb")
            nc.vector.memset(zf[:], 0.0)
            nc.vector.tensor_copy(out=zb[:], in_=zf[:])


            xb = [
                wp.tile([128, 2 * XROWS * PITCH], bf, name=f"xb{s}")
                for s in range(2)
            ]
            vt = [
                wp.tile([128, 2 * NK * VPLANE], bf, name=f"v{s}")
                for s in range(2)
            ]

            # matmuls on junk data keep the PE clock warm until real work
            # lands (~13us); vt[1] is only written from image 1 onward
            wm = pw.tile([128, PSW], f32, name="wm")
            for _ in range(48):
                nc.tensor.matmul(
                    wm[:, 0:NFREE], lhsT=zb[:, 0:128],
                    rhs=vt[1][:, 0:NFREE],
                    start=True, stop=True, skip_group_check=True,
                )

            def xin(t, img, cit, h0, hn):
                # SWDGE casts fp32->bf16 in the DMA datapath; packed rows
                src = xs[img, cit * 128 : (cit + 1) * 128].rearrange(
                    "p h w -> p (h w)"
                )[:, h0 * W : (h0 + hn) * W]
                return nc.gpsimd.dma_start(
                    out=t[:, cit, h0 * W : (h0 + hn) * W], in_=src
                )

            def xpk_tile():
                return xq.tile([128, 2, H * W], bf, name="xpk", tag="xpk")

            # image 0 arrives in wave-sized row chunks (both cin tiles per
            # chunk first) so transform and matmuls start as early as
            # possible; later chunks are dep-gated off the critical path
            QCH = ((0, 15), (13, 16), (27, 16), (41, 15))  # x-row chunks
            xpk0 = xpk_tile()
            q0ops = [xin(xpk0, 0, cit, *QCH[0]) for cit in range(2)]
            for cit in range(2):
                xin(xpk0, 0, cit, *QCH[1])
            # the first-wave weights release only after the q0 chunks land
            # so the transform inputs get the HBM bandwidth first
            for op in (
                nc.sync.dma_start(out=w_sb[:, 0, 0], in_=wt[0, 0]),
                nc.scalar.dma_start(out=w_sb[:, 1, 0], in_=wt[1, 0]),
            ):
                add_dep_helper(
                    op.ins, q0ops[1].ins, sync=False,
                    reason="input chunks get HBM bandwidth first",
                )

            # zero pads of the persistent xb tiles and V guard slots once;
            # slot 0 now (image 0 needs it), slot 1 off the critical path
            def init_slot(s):
                t4 = xb[s][:].rearrange(
                    "p (i r c) -> p i r c", r=XROWS, c=PITCH
                )
                nc.vector.tensor_copy(
                    out=t4[:, :, 0 : XROWS : XROWS - 1, :],
                    in_=zb[:, 0 : 4 * PITCH].rearrange(
                        "p (i a c) -> p i a c", i=2, c=PITCH
                    ),
                )
                z2 = zb[:, 0:232].rearrange("p (i a b) -> p i a b", i=2, b=2)
                nc.vector.tensor_copy(out=t4[:, :, 0:58, 56:58], in_=z2)
                nc.vector.tensor_copy(
                    out=vt[s][:].rearrange("p (g q) -> p g q", q=VPLANE)[
                        :, :, 0:2
                    ],
                    in_=zb[:, 0:16].rearrange("p (g b) -> p g b", b=2),
                )

            init_slot(0)

            def repitch(xpk, s, cit, h0, hn):
                # packed [56,56] -> pitch-58 rows as int32 pairs (4B aligned)
                d4 = xb[s][:].bitcast(i32).rearrange(
                    "p (i r c) -> p i r c", r=XROWS, c=PITCH // 2
                )
                s3 = xpk[:].bitcast(i32).rearrange(
                    "p i (r c) -> p i r c", c=W // 2
                )
                nc.vector.tensor_copy(
                    out=d4[:, cit, 1 + h0 : 1 + h0 + hn, 0 : W // 2],
                    in_=s3[:, cit, h0 : h0 + hn, :],
                )

            def vsl(s, cit, k, ty0, tn):
                r0 = (cit * NK + k) * VPLANE + 2 + ty0 * PITCH
                return vt[s][:, r0 : r0 + tn * PITCH].rearrange(
                    "p (t c) -> p t c", c=PITCH
                )

            def fwd(s, cit, ty0, tn):
                # V = BT d: 4 two-term combos, all +-1 coefficients
                x4 = xb[s][:].rearrange("p (i r c) -> p i r c", r=XROWS, c=PITCH)

                def dr(r):
                    lo = 2 * ty0 + r
                    return x4[:, cit, lo : lo + 2 * (tn - 1) + 1 : 2, :]

                tt = nc.vector.tensor_tensor
                tt(out=vsl(s, cit, 0, ty0, tn), in0=dr(0), in1=dr(2), op=SUB)
                tt(out=vsl(s, cit, 1, ty0, tn), in0=dr(1), in1=dr(2), op=ADD)
                tt(out=vsl(s, cit, 2, ty0, tn), in0=dr(2), in1=dr(1), op=SUB)
                return tt(
                    out=vsl(s, cit, 3, ty0, tn), in0=dr(1), in1=dr(3), op=SUB
                )

            def wino_ct(s, ct, m):
                vflat = vt[s]
                for ty0, tb in TYBLKS:
                    for k in range(NK):
                        ps = pp.tile([128, PSW], f32, name="ps", tag="ps")
                        idx = 0
                        for cit in range(2):
                            for kw in range(3):
                                st = (cit * NK + k) * VPLANE + 1 + ty0 * PITCH + kw
                                nc.tensor.matmul(
                                    ps[:, 0:NFREE],
                                    lhsT=w_sb[:, cit, ct, k, kw, :],
                                    rhs=vflat[:, st : st + NFREE],
                                    start=(idx == 0),
                                    stop=(idx == 5),
                                )
                                idx += 1
                        base = ((ct * NK + k) * TY + ty0) * PITCH
                        nc.scalar.copy(
                            out=m[:, base : base + NFREE], in_=ps[:, 0:NFREE]
                        )

            def inv(img, m):
                # y0 = m0+m1+m2 ; y1 = m1-m2-m3 (both cts, whole image)
                m5 = m[:].rearrange(
                    "p (i k t c) -> p i k t c", k=NK, t=TY, c=PITCH
                )
                tt = nc.vector.tensor_tensor
                u = tp.tile([128, 2 * TY * PITCH], f16, name="u", tag="tt")
                v = tp.tile([128, 2 * TY * PITCH], f16, name="v", tag="tt")
                u4 = u[:].rearrange("p (i t c) -> p i t c", t=TY, c=PITCH)
                v4 = v[:].rearrange("p (i t c) -> p i t c", t=TY, c=PITCH)
                tt(out=u4, in0=m5[:, :, 0], in1=m5[:, :, 1], op=ADD)
                tt(out=v4, in0=m5[:, :, 1], in1=m5[:, :, 2], op=SUB)
                for ct in range(2):
                    y = yp.tile([128, H * W], f16, name="y", tag="y")
                    yr = y[:].rearrange("p (r c) -> p r c", c=W)
                    tt(
                        out=yr[:, 0 : H : 2, :],
                        in0=u4[:, ct, :, 0:56],
                        in1=m5[:, ct, 2, :, 0:56],
                        op=ADD,
                    )
                    tt(
                        out=yr[:, 1 : H : 2, :],
                        in0=v4[:, ct, :, 0:56],
                        in1=m5[:, ct, 3, :, 0:56],
                        op=SUB,
                    )
                    # SWDGE casts fp16->fp32 on the way out
                    nc.gpsimd.dma_start(
                        out=out[img, ct * 128 : (ct + 1) * 128].rearrange(
                            "p h w -> p (h w)"
                        ),
                        in_=y[:],
                    )

            def inv_part(img, m, ct, ty0, tn, y):
                # per-ct, per-ty-range inverse + store (tail overlap)
                m5 = m[:].rearrange(
                    "p (i k t c) -> p i k t c", k=NK, t=TY, c=PITCH
                )
                tt = nc.vector.tensor_tensor
                u = tp.tile([128, 2 * TY * PITCH], f16, name="u", tag="tt")
                u4 = u[:].rearrange("p (i t c) -> p i t c", t=TY, c=PITCH)
                ts = slice(ty0, ty0 + tn)
                yr = y[:].rearrange("p (r c) -> p r c", c=W)
                tt(out=u4[:, 0, ts, :], in0=m5[:, ct, 0, ts, :],
                   in1=m5[:, ct, 1, ts, :], op=ADD)
                tt(out=u4[:, 1, ts, :], in0=m5[:, ct, 1, ts, :],
                   in1=m5[:, ct, 2, ts, :], op=SUB)
                tt(out=yr[:, 2 * ty0 : 2 * (ty0 + tn) : 2, :],
                   in0=u4[:, 0, ts, 0:56], in1=m5[:, ct, 2, ts, 0:56], op=ADD)
                tt(out=yr[:, 2 * ty0 + 1 : 2 * (ty0 + tn) : 2, :],
                   in0=u4[:, 1, ts, 0:56], in1=m5[:, ct, 3, ts, 0:56], op=SUB)
                nc.gpsimd.dma_start(
                    out=out[img, ct * 128 : (ct + 1) * 128].rearrange(
                        "p h w -> p (h w)"
                    )[:, 2 * ty0 * W : 2 * (ty0 + tn) * W],
                    in_=y[:, 2 * ty0 * W : 2 * (ty0 + tn) * W],
                )

            # ---- pipeline ----
            pending = []
            xpks = {0: xpk0}
            last = B_PER_CORE - 1
            for img in range(B_PER_CORE):
                s = img % 2
                lastf = None
                if img == 0:
                    for q in range(4):
                        h0, hn = QCH[q]
                        for cit in range(2):
                            repitch(xpks[0], s, cit, h0, hn)
                            lastf = fwd(s, cit, 7 * q, 7)
                        if q == 0:
                            # critical loads done: release the lower-half
                            # chunks and the ct=1 weights
                            for op in (
                                xin(xpk0, 0, 0, *QCH[2]),
                                xin(xpk0, 0, 1, *QCH[2]),
                                xin(xpk0, 0, 0, *QCH[3]),
                                xin(xpk0, 0, 1, *QCH[3]),
                                nc.sync.dma_start(
                                    out=w_sb[:, 0, 1], in_=wt[0, 1]
                                ),
                                nc.scalar.dma_start(
                                    out=w_sb[:, 1, 1], in_=wt[1, 1]
                                ),
                            ):
                                add_dep_helper(
                                    op.ins, lastf.ins, sync=False,
                                    reason="keep head HBM bw for critical loads",
                                )
                    init_slot(1)
                else:
                    for cit in range(2):
                        repitch(xpks[img], s, cit, 0, H)
                        lastf = fwd(s, cit, 0, TY)
                if img + 1 < B_PER_CORE:
                    t = xpk_tile()
                    for cit in range(2):
                        op = xin(t, img + 1, cit, 0, H)
                        if img == 0:
                            add_dep_helper(
                                op.ins, lastf.ins, sync=False,
                                reason="keep head HBM bw for critical loads",
                            )
                    xpks[img + 1] = t
                m = mp.tile([128, 2 * NK * TY * PITCH], f16, name="m", tag="m")
                if img == last:
                    wino_ct(s, 0, m)
                    while pending:
                        inv(*pending.pop(0))
                    y0t = yp.tile([128, H * W], f16, name="y", tag="y")
                    inv_part(img, m, 0, 0, TY, y0t)
                    wino_ct(s, 1, m)
                    y1t = yp.tile([128, H * W], f16, name="y", tag="y")
                    for ty0, tn in TYBLKS:
                        inv_part(img, m, 1, ty0, tn, y1t)
                else:
                    for ct in range(2):
                        wino_ct(s, ct, m)
                    pending.append((img, m))
                    while len(pending) > 1:
                        inv(*pending.pop(0))

    nc.compile()
    return nc


def _get_nc():
    if "nc" not in _CACHED:
        _CACHED["nc"] = _build_nc()
    return _CACHED["nc"]


_G = np.array(
    [[1, 0, 0], [0.5, 0.5, 0.5], [0.5, -0.5, 0.5], [0, 0, 1]],
    dtype=np.float64,
)


def _prep_weights(W_arr):
    import ml_dtypes

    Wb = np.sign(np.asarray(W_arr, dtype=np.float64))  # [co, ci, kh, kw]
    U = np.einsum("kh,oihw->koiw", _G, Wb)  # [4, co, ci, kw]
    U6 = U.reshape(NK, 2, 128, 2, 128, 3)  # k, ct, co, cit, ci, kw
    wt = U6.transpose(3, 1, 4, 0, 5, 2)  # cit, ct, ci, k, kw, co
    return np.ascontiguousarray(wt).astype(ml_dtypes.bfloat16)


def run(x, W, trace=False, trace_kwargs=None):
    from concourse.bass_utils import run_bass_kernel_spmd

    x = np.asarray(x, dtype=np.float32)
    wt = _prep_weights(W)
    nc = _get_nc()
    in_maps = [
        {
            "xs": np.ascontiguousarray(
                x[i * B_PER_CORE : (i + 1) * B_PER_CORE]
            ),
            "wt": wt,
        }
        for i in range(N_CORES)
    ]
    res = run_bass_kernel_spmd(
        nc,
        in_maps,
        list(range(N_CORES)),
        trace=trace,
        trace_kwargs=trace_kwargs or {},
    )
    out = np.concatenate(
        [np.asarray(res.results[i]["out"]) for i in range(N_CORES)]
    )
    return out, res


def kernel(x, W):
    out, _ = run(x, W, trace=False)
    return out
